# revision 4
# baseline (speedup 1.0000x reference)
import sys

sys.path.insert(0, "/opt/trn_rl_repo")

import numpy as np

NCORES = 8
B, FULL_N, D = 4, 2048, 1024
NH = 16
DK = 64  # head dim
HPC = NH // NCORES  # heads per core = 2
CW = HPC * DK  # output columns per core = 128
DC = D // 128  # D chunks = 8

_CACHE = {}
LAST_RESULTS = None


def _build(n_rows):
    """Build the SPMD Bass program for one core (parameterized row count for
    fast small-scale testing). Each core computes batch-0 attention for its 2
    heads (the reference only uses att[0]) and adds it to its column slice of
    tgt for all batches."""
    import concourse.mybir as mybir
    import concourse.tile as tile
    from concourse import bacc
    from concourse.masks import make_identity

    fp32 = mybir.dt.float32
    fp32r = mybir.dt.float32r

    RT = n_rows // 128  # row tiles
    G = n_rows // 512  # 512-row groups
    QG = G  # q groups (512 wide)
    KC = RT  # k chunks of 128

    nc = bacc.Bacc(None, target_bir_lowering=False)
    tgt0 = nc.declare_dram_parameter("tgt0", [n_rows, D], fp32, isOutput=False)
    mem0 = nc.declare_dram_parameter("mem0", [n_rows, D], fp32, isOutput=False)
    wq = nc.declare_dram_parameter("wq", [CW, D], fp32, isOutput=False)
    wk = nc.declare_dram_parameter("wk", [CW, D], fp32, isOutput=False)
    wv = nc.declare_dram_parameter("wv", [CW, D], fp32, isOutput=False)
    tgtc = nc.declare_dram_parameter("tgtc", [B, n_rows, CW], fp32, isOutput=False)
    outc = nc.declare_dram_parameter("outc", [B, n_rows, CW], fp32, isOutput=True)

    Exp = mybir.ActivationFunctionType.Exp
    Copy = mybir.ActivationFunctionType.Copy
    scale = 1.0 / np.sqrt(DK)

    with tile.TileContext(nc) as tc:
        with (
            tc.tile_pool(name="const", bufs=1) as const,
            tc.tile_pool(name="persist", bufs=1) as persist,
        ):
            ident = const.tile([128, 128], fp32)
            make_identity(nc, ident)

            # persistent SBUF tensors
            QT = persist.tile([128, n_rows], fp32r, tag="QT")  # [qdim(2 heads), rows]
            KTs = persist.tile([128, n_rows], fp32r, tag="KTs")  # [kdim, rows]
            # V with appended ones column: partition = row-in-chunk, free = (head, chunk, dk+1)
            Vp2 = persist.tile([128, HPC, KC, DK + 1], fp32r, tag="Vp2")
            att_sb = persist.tile([128, RT, CW], fp32, tag="att")
            tgtc_sb = persist.tile([128, B, RT, CW], fp32, tag="tgtc")

            # load tgt columns early (overlaps with compute)
            for b in range(B):
                nc.sync.dma_start(
                    out=tgtc_sb[:, b, :, :],
                    in_=tgtc[b, :, :].rearrange("(t p) c -> p t c", p=128),
                )

            # ---------------- Phase A: transposes + QKV projections -----------
            with (
                tc.tile_pool(name="stage", bufs=3) as stage_pool,
                tc.tile_pool(name="wtile", bufs=1) as wpool,
                tc.tile_pool(name="grp", bufs=2) as grp_pool,
                tc.tile_pool(name="vtg", bufs=2) as vt_pool,
                tc.tile_pool(name="ps_tr", bufs=4, space="PSUM") as ps_tr,
                tc.tile_pool(name="ps_acc", bufs=3, space="PSUM") as ps_acc,
            ):
                # weight transposes: W [CW, D] -> WT [128(Dchunk), dchunks, CW]
                WTs = {}
                for name, w in (("q", wq), ("k", wk), ("v", wv)):
                    wst = wpool.tile([CW, D], fp32, tag=f"wst{name}")
                    nc.sync.dma_start(out=wst, in_=w[:, :])
                    wt = wpool.tile([128, DC, CW], fp32r, tag=f"wt{name}")
                    for d in range(DC):
                        ptr = ps_tr.tile([128, 128], fp32, tag="tr")
                        nc.tensor.transpose(ptr, wst[:, d * 128 : (d + 1) * 128], ident)
                        eng = nc.vector if d % 2 == 0 else nc.scalar
                        if eng is nc.vector:
                            nc.vector.tensor_copy(out=wt[:, d, :], in_=ptr[:, 0:CW])
                        else:
                            nc.scalar.activation(out=wt[:, d, :], in_=ptr[:, 0:CW], func=Copy)
                    WTs[name] = wt

                cp = 0  # copy-engine round robin counter
                for g in range(G):
                    # --- tgt rows group -> tgt0T_g, then QT columns ---
                    tgtT_g = grp_pool.tile([128, DC, 512], fp32r, tag="tgtTg")
                    for t in range(4):
                        r = g * 4 + t
                        st = stage_pool.tile([128, D], fp32, tag="tstage")
                        nc.sync.dma_start(out=st, in_=tgt0[r * 128 : (r + 1) * 128, :])
                        for d in range(DC):
                            ptr = ps_tr.tile([128, 128], fp32, tag="tr")
                            nc.tensor.transpose(ptr, st[:, d * 128 : (d + 1) * 128], ident)
                            dst = tgtT_g[:, d, t * 128 : (t + 1) * 128]
                            if cp % 2 == 0:
                                nc.vector.tensor_copy(out=dst, in_=ptr)
                            else:
                                nc.scalar.activation(out=dst, in_=ptr, func=Copy)
                            cp += 1
                    pq = ps_acc.tile([128, 512], fp32, tag="acc")
                    for d in range(DC):
                        nc.tensor.matmul(
                            pq,
                            WTs["q"][:, d, :],
                            tgtT_g[:, d, :],
                            start=(d == 0),
                            stop=(d == DC - 1),
                        )
                    nc.vector.tensor_copy(out=QT[:, g * 512 : (g + 1) * 512], in_=pq)

                    # --- memory rows group -> memT_g, then KT + VT + V ---
                    memT_g = grp_pool.tile([128, DC, 512], fp32r, tag="memTg")
                    for t in range(4):
                        r = g * 4 + t
                        st = stage_pool.tile([128, D], fp32, tag="mstage")
                        nc.sync.dma_start(out=st, in_=mem0[r * 128 : (r + 1) * 128, :])
                        for d in range(DC):
                            ptr = ps_tr.tile([128, 128], fp32, tag="tr")
                            nc.tensor.transpose(ptr, st[:, d * 128 : (d + 1) * 128], ident)
                            dst = memT_g[:, d, t * 128 : (t + 1) * 128]
                            if cp % 2 == 0:
                                nc.vector.tensor_copy(out=dst, in_=ptr)
                            else:
                                nc.scalar.activation(out=dst, in_=ptr, func=Copy)
                            cp += 1
                    pk = ps_acc.tile([128, 512], fp32, tag="acc")
                    for d in range(DC):
                        nc.tensor.matmul(
                            pk,
                            WTs["k"][:, d, :],
                            memT_g[:, d, :],
                            start=(d == 0),
                            stop=(d == DC - 1),
                        )
                    nc.scalar.activation(
                        out=KTs[:, g * 512 : (g + 1) * 512], in_=pk, func=Copy
                    )
                    pv = ps_acc.tile([128, 512], fp32, tag="acc")
                    for d in range(DC):
                        nc.tensor.matmul(
                            pv,
                            WTs["v"][:, d, :],
                            memT_g[:, d, :],
                            start=(d == 0),
                            stop=(d == DC - 1),
                        )
                    vt_g = vt_pool.tile([128, 512], fp32, tag="vtg")
                    nc.vector.tensor_copy(out=vt_g, in_=pv)
                    # VT -> V (rows on partitions), split per head, append ones col
                    for t in range(4):
                        j = g * 4 + t
                        ptr = ps_tr.tile([128, 128], fp32, tag="tr")
                        nc.tensor.transpose(ptr, vt_g[:, t * 128 : (t + 1) * 128], ident)
                        nc.vector.tensor_copy(
                            out=Vp2[:, 0, j, 0:DK], in_=ptr[:, 0:DK]
                        )
                        nc.scalar.activation(
                            out=Vp2[:, 1, j, 0:DK], in_=ptr[:, DK : 2 * DK], func=Copy
                        )
                ones_f32 = wpool.tile([128, HPC, KC], fp32, tag="ones")
                nc.vector.memset(ones_f32, 1.0)
                nc.vector.tensor_copy(out=Vp2[:, :, :, DK], in_=ones_f32)

            # ---------------- Phase B: attention per (head, q-group) ----------
            with (
                tc.tile_pool(name="pt", bufs=2) as pt_pool,
                tc.tile_pool(name="usb", bufs=2) as usb_pool,
                tc.tile_pool(name="small", bufs=8) as small_pool,
                tc.tile_pool(name="ps_st", bufs=4, space="PSUM") as ps_st,
                tc.tile_pool(name="ps_u", bufs=2, space="PSUM") as ps_u,
                tc.tile_pool(name="ps_t", bufs=2, space="PSUM") as ps_t,
            ):
                for qg in range(QG):
                    for h in range(HPC):
                        hs = h * DK
                        pt = pt_pool.tile([128, KC, 512], fp32r, tag="pt")
                        for j in range(KC):
                            pst = ps_st.tile([128, 512], fp32, tag="st")
                            nc.tensor.matmul(
                                pst,
                                KTs[hs : hs + DK, j * 128 : (j + 1) * 128],
                                QT[hs : hs + DK, qg * 512 : (qg + 1) * 512],
                                start=True,
                                stop=True,
                            )
                            nc.scalar.activation(
                                out=pt[:, j, :], in_=pst, func=Exp, scale=float(scale)
                            )
                        pu = ps_u.tile([DK + 1, 512], fp32, tag="u")
                        for j in range(KC):
                            nc.tensor.matmul(
                                pu,
                                Vp2[:, h, j, :],
                                pt[:, j, :],
                                start=(j == 0),
                                stop=(j == KC - 1),
                            )
                        pu_sb = usb_pool.tile([DK + 1, 512], fp32, tag="usb")
                        nc.vector.tensor_copy(out=pu_sb, in_=pu)
                        for s in range(4):
                            pat = ps_t.tile([128, DK + 1], fp32, tag="t")
                            nc.tensor.transpose(
                                pat,
                                pu_sb[:, s * 128 : (s + 1) * 128],
                                ident[0 : DK + 1, 0 : DK + 1],
                            )
                            rec = small_pool.tile([128, 1], fp32, tag="rec")
                            nc.vector.reciprocal(rec, pat[:, DK : DK + 1])
                            nc.vector.tensor_scalar_mul(
                                att_sb[:, qg * 4 + s, hs : hs + DK],
                                in0=pat[:, 0:DK],
                                scalar1=rec,
                            )

            # ---------------- Final: out[b] = tgt_cols[b] + att ----------------
            for b in range(B):
                nc.vector.tensor_add(
                    out=tgtc_sb[:, b, :, :], in0=tgtc_sb[:, b, :, :], in1=att_sb
                )
                nc.sync.dma_start(
                    out=outc[b, :, :].rearrange("(t p) c -> p t c", p=128),
                    in_=tgtc_sb[:, b, :, :],
                )

    nc.finalize()
    return nc


def _get_nc(n_rows):
    if n_rows not in _CACHE:
        _CACHE[n_rows] = _build(n_rows)
    return _CACHE[n_rows]


def _run(tgt, memory, Wq, Wk, Wv, trace=False):
    global LAST_RESULTS
    from concourse.bass_utils import run_bass_kernel_spmd

    n_rows = tgt.shape[1]
    nc = _get_nc(n_rows)

    tgt = np.ascontiguousarray(tgt, dtype=np.float32)
    memory = np.ascontiguousarray(memory, dtype=np.float32)
    tgt0 = np.ascontiguousarray(tgt[0])
    mem0 = np.ascontiguousarray(memory[0])

    in_maps = []
    for c in range(NCORES):
        sl = slice(c * CW, (c + 1) * CW)
        in_maps.append(
            {
                "tgt0": tgt0,
                "mem0": mem0,
                "wq": np.ascontiguousarray(Wq[sl, :], dtype=np.float32),
                "wk": np.ascontiguousarray(Wk[sl, :], dtype=np.float32),
                "wv": np.ascontiguousarray(Wv[sl, :], dtype=np.float32),
                "tgtc": np.ascontiguousarray(tgt[:, :, sl]),
            }
        )
    res = run_bass_kernel_spmd(nc, in_maps, list(range(NCORES)), trace=trace)
    LAST_RESULTS = res
    out = np.concatenate([res.results[c]["outc"] for c in range(NCORES)], axis=2)
    return out


def kernel(tgt, memory, Wq, Wk, Wv):
    return _run(tgt, memory, Wq, Wk, Wv)
